# revision 6
# baseline (speedup 1.0000x reference)
"""Trainium2 Bass kernel for nn_AutoIntTPPSameInfluence — exp-sum formulation.

dF(x) (the scalar derivative of the 1->64->64->64->1 tanh MLP) is fit on host
as a sum of 8 decaying exponentials  dF(x) ~= sum_i c_i exp(-s_i x)  (ridge
LSQ over a geomspace rate grid).  On device every pairwise influence point is
then ONE table-exp evaluation: points are packed 16-per-segment-column and
replicated over the 8 partition groups of a [128, N] tile, a single ACT
instruction computes |c_i| exp(-s_i x + ln|c_i|) for all 8 rates via
per-partition scale/bias, and one bf16 matmul with a +-1 selector contracts
all 128 partitions — performing the 16-point segment sum AND the 8-term
weighted sum at 1 col/cycle.  Masked/padded points use x = 3e4, driving every
exponential to exactly 0.

The integral term sum_k F(T_END - t_k) - F0 only needs a bulk sum, so
F(x) - F0 is fit the same way (const + 8 exps) and rides through the same
pipeline as one extra tile; the constant is applied on host via the valid
count.  Host: scatter segment sums to events, log/mask/reduce in float64.
"""

import numpy as np
from contextlib import ExitStack

import ml_dtypes

import concourse.bass as bass
import concourse.bacc as bacc
import concourse.tile as tile
import concourse.mybir as mybir
from concourse.bass_utils import run_bass_kernel_spmd

B, L, H = 16, 320, 64
T_END = 100.0
NC = 8
SEG = 16                    # points per segment column
K = 8                       # exponential rates (K * SEG == 128)
TS_MAX = 512                # PSUM bank width in f32
XPAD = np.float32(30000.0)  # pad x: exp(-s*XPAD) underflows to exactly 0
BF16 = mybir.dt.bfloat16
F32 = mybir.dt.float32
Exp = mybir.ActivationFunctionType.Exp
NPBF16 = ml_dtypes.bfloat16


# ---------------------------------------------------------------- host fits
_FIT_CACHE = {}


def _mlp_funcs(W1, b1, W2, b2, W3, b3, W4, b4):
    w1 = W1[:, 0].astype(np.float64)
    b1d, b2d, b3d = (b1.astype(np.float64), b2.astype(np.float64),
                     b3.astype(np.float64))
    W2d, W3d, W4d = (W2.astype(np.float64), W3.astype(np.float64),
                     W4.astype(np.float64))
    b4d = float(np.asarray(b4, np.float64)[0])

    def dF(x):
        x = np.ravel(x)
        h1 = np.outer(w1, x) + b1d[:, None]
        a1 = np.tanh(h1)
        d1 = (1 - a1 ** 2) * w1[:, None]
        h2 = W2d @ a1 + b2d[:, None]
        a2 = np.tanh(h2)
        d2 = (1 - a2 ** 2) * (W2d @ d1)
        h3 = W3d @ a2 + b3d[:, None]
        a3 = np.tanh(h3)
        d3 = (1 - a3 ** 2) * (W3d @ d2)
        return (W4d @ d3)[0]

    def F(x):
        x = np.ravel(x)
        h1 = np.tanh(np.outer(w1, x) + b1d[:, None])
        h2 = np.tanh(W2d @ h1 + b2d[:, None])
        h3 = np.tanh(W3d @ h2 + b3d[:, None])
        return (W4d @ h3)[0] + b4d

    return dF, F


def _ridge_fit(A, y, lam):
    cn = np.linalg.norm(A, axis=0)
    return np.linalg.solve(A.T @ A + lam * np.diag(cn ** 2), A.T @ y)


def _fits(W1, b1, W2, b2, W3, b3, W4, b4):
    key = b"".join(np.ascontiguousarray(a).tobytes()
                   for a in (W1, b1, W2, b2, W3, b3, W4, b4))
    if key in _FIT_CACHE:
        return _FIT_CACHE[key]
    dF, F = _mlp_funcs(W1, b1, W2, b2, W3, b3, W4, b4)
    F0 = float(F(np.zeros(1))[0])

    # dF: relative-weighted fit with absolute floor; scan geomspace rate grids
    gx = np.unique(np.concatenate([np.geomspace(1e-4, 100, 6000),
                                   np.linspace(0, 100, 6000)]))
    gy = dF(gx)
    wts = 1.0 / (np.abs(gy) + 2e-4)
    best = None
    for smin in (0.02, 0.04, 0.08, 0.15):
        for smax in (10.0, 13.0, 16.0, 20.0, 25.0):
            r = np.geomspace(smin, smax, K)
            A = np.exp(-np.outer(gx, r)) * wts[:, None]
            c = _ridge_fit(A, gy * wts, 1e-5)
            if np.abs(c).max() > 300.0:
                continue
            werr = np.abs((np.exp(-np.outer(gx, r)) @ c - gy) * wts).max()
            if best is None or werr < best[0]:
                best = (werr, r, c)
    _, rates, c = best

    # F - F0: absolute fit (const + exps), uniform grid
    gxF = np.linspace(0, 100, 20001)
    gyF = F(gxF) - F0
    ratesF = np.geomspace(0.04, 16.0, K)
    AF = np.concatenate([np.ones((len(gxF), 1)),
                         np.exp(-np.outer(gxF, ratesF))], axis=1)
    cf = _ridge_fit(AF, gyF, 1e-7)
    CF, cF = float(cf[0]), cf[1:]

    out = (rates, c, ratesF, cF, CF, F0)
    _FIT_CACHE[key] = out
    return out


# ---------------------------------------------------------------- packing
def _pack(t, lens):
    """-> xrow [NC, 16, XC] f32, seg_ev [NC, NTS*SEGT], (NTS, SEGT, FT)."""
    bs, ks = [], []
    for b in range(B):
        n = int(lens[b])
        ksb = np.arange(1, n, dtype=np.int64)
        ks.append(ksb)
        bs.append(np.full_like(ksb, b))
    bs = np.concatenate(bs)
    ks = np.concatenate(ks)
    nseg = (ks + SEG - 1) // SEG
    Tseg = int(nseg.sum())

    Spc = (Tseg + NC - 1) // NC
    NTS = (Spc + TS_MAX - 1) // TS_MAX
    SEGT = (Spc + NTS - 1) // NTS
    SEGT = (SEGT + 7) // 8 * 8
    Gc = NTS * SEGT                   # seg columns per core
    G = NC * Gc

    seg_b = np.zeros(G, np.int64)
    seg_k = np.zeros(G, np.int64)
    seg_j0 = np.zeros(G, np.int64)
    seg_ev = np.full(G, -1, np.int64)
    ev_idx = np.repeat(np.arange(len(ks)), nseg)
    seg_b[:Tseg] = bs[ev_idx]
    seg_k[:Tseg] = ks[ev_idx]
    seg_ev[:Tseg] = seg_b[:Tseg] * L + seg_k[:Tseg]
    starts = np.concatenate([[0], np.cumsum(nseg)[:-1]])
    seg_j0[:Tseg] = (np.arange(Tseg) - np.repeat(starts, nseg)) * SEG

    jj = seg_j0[:, None] + np.arange(SEG)[None, :]
    valid = jj < seg_k[:, None]
    valid[Tseg:] = False
    jc = np.minimum(jj, L - 1)
    x = np.where(valid,
                 (t[seg_b[:, None], jc] * -1.0 + t[seg_b, seg_k][:, None]),
                 XPAD).astype(np.float32)                  # [G, SEG]
    xs = x.reshape(NC, Gc, SEG).transpose(0, 2, 1)         # [NC, 16, Gc]

    # F points: one per valid event (all k < n)
    fb, fk = [], []
    for b in range(B):
        n = int(lens[b])
        fk.append(np.arange(n, dtype=np.int64))
        fb.append(np.full(n, b, np.int64))
    fb = np.concatenate(fb)
    fk = np.concatenate(fk)
    nF = len(fk)
    nFc = (nF + NC - 1) // NC
    FT = ((nFc + SEG - 1) // SEG + 1) // 2 * 2
    xf = np.full((NC * FT * SEG,), XPAD, np.float32)
    xf[:nF] = (T_END - t[fb, fk]).astype(np.float32)
    xf = xf.reshape(NC, FT, SEG).transpose(0, 2, 1)        # [NC, 16, FT]

    assert SEGT + FT <= TS_MAX
    xrow = np.concatenate([xs, xf], axis=2)                # [NC, 16, XC]
    return xrow, seg_ev.reshape(NC, Gc), (NTS, SEGT, FT), nF


# ---------------------------------------------------------------- program
_PROGRAM_CACHE = {}


def build_program(NTS, SEGT, FT):
    pkey = (NTS, SEGT, FT)
    if pkey in _PROGRAM_CACHE:
        return _PROGRAM_CACHE[pkey]
    XC = NTS * SEGT + FT
    NG = (NTS + 7) // 8
    CHT = 2                                  # tiles per ACT/DMA chunk
    nc = bacc.Bacc("TRN2", target_bir_lowering=False, debug=False,
                   enable_asserts=False)

    xbb_d = nc.dram_tensor("xbb", [128, XC], BF16, kind="ExternalInput")
    selv_d = nc.dram_tensor("selv", [128, 8 * 8 + 8], BF16,
                            kind="ExternalInput")
    cf_d = nc.dram_tensor("cfd", [128, 4], F32, kind="ExternalInput")
    outs_d = nc.dram_tensor("out_s", [NG, 8, SEGT], F32,
                            kind="ExternalOutput")
    outf_d = nc.dram_tensor("out_f", [8, FT], F32, kind="ExternalOutput")

    with tile.TileContext(nc) as tc, ExitStack() as ctx, \
            nc.allow_low_precision(reason="bf16 exp terms; tol is 2e-2"):
        consts = ctx.enter_context(tc.tile_pool(name="consts", bufs=1))
        xb_p = ctx.enter_context(tc.tile_pool(name="xb", bufs=3))
        term_p = ctx.enter_context(tc.tile_pool(name="term", bufs=3))
        outp_p = ctx.enter_context(tc.tile_pool(name="outp", bufs=2,
                                                space="PSUM"))
        outf_p = ctx.enter_context(tc.tile_pool(name="outf", bufs=1,
                                                space="PSUM"))
        stage_p = ctx.enter_context(tc.tile_pool(name="stage", bufs=2))

        sel_raw = consts.tile([128, 72], BF16, tag="selraw")
        nc.gpsimd.dma_start(out=sel_raw[:], in_=selv_d.ap())
        selc = consts.tile([128, 72], BF16, tag="selc")
        nc.vector.tensor_copy(selc[:], sel_raw[:])
        cf_raw = consts.tile([128, 4], F32, tag="cfraw")
        nc.gpsimd.dma_start(out=cf_raw[:], in_=cf_d.ap())
        cfc = consts.tile([128, 4], F32, tag="cfc")
        nc.vector.tensor_copy(cfc[:], cf_raw[:])

        # chunked input DMA so ACT can start before the whole tensor lands
        bounds = list(range(0, NTS, CHT)) + [NTS]
        NCH = len(bounds) - 1
        xbts = []
        for ci in range(NCH):
            a = bounds[ci] * SEGT
            z = bounds[ci + 1] * SEGT + (FT if ci == NCH - 1 else 0)
            xbt = xb_p.tile([128, z - a], BF16, tag="xb")
            nc.sync.dma_start(out=xbt[:], in_=xbb_d.ap()[:, a:z])
            xbts.append(xbt)

        terms = []
        for ci in range(NCH):
            w = (bounds[ci + 1] - bounds[ci]) * SEGT
            tt = term_p.tile([128, w], BF16, tag="terms")
            nc.scalar.activation(tt[:], xbts[ci][:, :w], Exp,
                                 bias=cfc[:, 1:2], scale=cfc[:, 0:1])
            terms.append((bounds[ci], tt))

        outbs = {}
        for t0, tt in terms:
            for j in range(CHT):
                tl = t0 + j
                if tl >= NTS:
                    break
                g, t8 = tl // 8, tl % 8
                if t8 == 0:
                    outbs[g] = outp_p.tile([8, SEGT], F32, tag="outb",
                                           name=f"outb{g}")
                in_g = min(8, NTS - 8 * g)
                nc.tensor.matmul(out=outbs[g][:],
                                 lhsT=selc[:, 8 * t8:8 * t8 + 8],
                                 rhs=tt[:, j * SEGT:(j + 1) * SEGT],
                                 start=(t8 == 0), stop=(t8 == in_g - 1))
                if t8 == in_g - 1:
                    st = stage_p.tile([8, SEGT], F32, tag="st")
                    nc.vector.tensor_copy(st[:], outbs[g][:])
                    nc.gpsimd.dma_start(out=outs_d.ap()[g], in_=st[:])

        termf = term_p.tile([128, FT], BF16, tag="termf")
        wlast = (bounds[NCH] - bounds[NCH - 1]) * SEGT
        nc.scalar.activation(termf[:], xbts[-1][:, wlast:], Exp,
                             bias=cfc[:, 3:4], scale=cfc[:, 2:3])
        outf = outf_p.tile([8, FT], F32, tag="outf")
        nc.tensor.matmul(out=outf[:], lhsT=selc[:, 64:72], rhs=termf[:],
                         start=True, stop=True)
        stf = stage_p.tile([8, FT], F32, tag="stf")
        nc.vector.tensor_copy(stf[:], outf[:])
        nc.gpsimd.dma_start(out=outf_d.ap(), in_=stf[:])

    nc.compile()
    prog = (nc, pkey)
    _PROGRAM_CACHE[pkey] = prog
    return prog


# ---------------------------------------------------------------- kernel
def kernel(seq_pads, background, W1, b1, W2, b2, W3, b3, W4, b4, seq_lens):
    t = np.asarray(seq_pads)[:, :, 0].astype(np.float32)
    lens = np.asarray(seq_lens).astype(np.int64)
    rates, c, ratesF, cF, CF, F0 = _fits(
        np.asarray(W1, np.float64), np.asarray(b1, np.float64),
        np.asarray(W2, np.float64), np.asarray(b2, np.float64),
        np.asarray(W3, np.float64), np.asarray(b3, np.float64),
        np.asarray(W4, np.float64), np.asarray(b4, np.float64))

    xrow, seg_ev, (NTS, SEGT, FT), nF = _pack(t, lens)
    nc, _ = build_program(NTS, SEGT, FT)

    grp = np.repeat(np.arange(K), SEG)                     # partition -> rate
    cfd = np.zeros((128, 4), np.float32)
    cfd[:, 0] = -rates[grp]
    cfd[:, 1] = np.log(np.maximum(np.abs(c), 1e-20))[grp]
    cfd[:, 2] = -ratesF[grp]
    cfd[:, 3] = np.log(np.maximum(np.abs(cF), 1e-20))[grp]
    selv = np.zeros((128, 72), np.float32)
    sgn = np.sign(c)[grp]
    for v in range(8):
        selv[:, 8 * v + v] = sgn
    selv[:, 64] = np.sign(cF)[grp]
    selv = selv.astype(NPBF16)
    cs = dict(selv=selv, cfd=cfd)

    in_maps = []
    for cix in range(NC):
        m = dict(cs)
        m["xbb"] = np.ascontiguousarray(
            np.tile(xrow[cix], (K, 1)).astype(NPBF16))
        in_maps.append(m)

    res = run_bass_kernel_spmd(nc, in_maps, core_ids=list(range(NC))).results
    if any(not np.isfinite(res[cc][k]).all() for cc in range(NC)
           for k in ("out_s", "out_f")):
        res = run_bass_kernel_spmd(nc, in_maps, core_ids=list(range(NC))).results

    Gc = NTS * SEGT
    partials = np.concatenate(
        [res[cc]["out_s"].reshape(-1)[:Gc] for cc in range(NC)])
    f_exp = float(sum(res[cc]["out_f"].sum() for cc in range(NC)))

    S = np.zeros(B * L, np.float64)
    ok = seg_ev.reshape(-1) >= 0
    np.add.at(S, seg_ev.reshape(-1)[ok], partials[ok].astype(np.float64))
    S = S.reshape(B, L)

    bg = float(np.asarray(background)[0])
    lam = bg + S
    mask = np.arange(L)[None, :] < lens[:, None]
    sum_log = np.log(np.where(mask, lam, 1.0)).sum()

    ints_total = f_exp + nF * CF + B * T_END * bg
    nll = -(sum_log - ints_total) / B
    return np.float32(nll)
